# revision 6
# baseline (speedup 1.0000x reference)
"""Trainium2 Bass kernel for BoundaryFeaturePropagation (v2).

Sharding: data-parallel over batch — one image per NeuronCore (B=8 over
8 cores); the small [C,C] weights are replicated on all cores.

Per-core pipeline:
  1. gate:   conf = clip(1 - beta*sigmoid(a*sigmoid(bl) - g), 0, 1); PE
             transposes build per-step gate rows; staged to DRAM and
             broadcast back one row per step (stride-0 partition reads).
  2. scan:   4 directional gated RNN scans, state layout [c(part), n(free)],
             combined PSUM tile [128, 2(m), 4(d), PBLK*S].
             - input proj: 3-term error-compensated fp8 (W8@x8 + W8@xlo8 +
               Wlo8@x8), each matmul in DoubleRow perf mode (2 k-subtiles
               contracted per instruction at 0.5 cyc/row).
             - state matmuls: bf16, per (d, m, k).
             - per-dir h=relu(ps) evac into a hist ring (ACT for dirs 0-2,
               DVE tensor_scalar for dir 3), then hg = h*g as bf16 SBUF
               tensor_tensor (DVE 2x mode), per direction, so each
               direction's recurrence chain pipelines independently.
             - acc += hist block-adds every ABLK steps: lr/rl on gpsimd
               (STT form), tb/bt on DVE (packed, 2x mode).
  3. output: single matmul pass Wo@acc -> z stored bf16 (resident), with
             residual x re-streamed from DRAM; GN stats via accum_out on
             the z ops; second lightweight pass applies the folded GN
             affine (DVE tensor_scalar, 4x mode) and streams out bf16.
"""

import os
import sys

for _p in ("/opt/trn_rl_repo",):
    if _p not in sys.path and os.path.isdir(_p):
        sys.path.insert(0, _p)

import numpy as np
import ml_dtypes
from contextlib import ExitStack

import concourse.bass as bass
import concourse.bacc as bacc
import concourse.mybir as mybir
import concourse.tile as tile
from concourse.bass_utils import run_bass_kernel_spmd

BF = ml_dtypes.bfloat16
F8 = ml_dtypes.float8_e4m3fn
F32 = mybir.dt.float32
DBF = mybir.dt.bfloat16
DF8 = mybir.dt.float8e4
AF = mybir.ActivationFunctionType
OP = mybir.AluOpType
DR = mybir.MatmulPerfMode.DoubleRow

ALPHA = 20.0
GAMMA = 4.0
GN_GROUPS = 32
EPS = 1e-5

C = 256
NK = 2          # c-halves
S = 128         # H = W
HW = S * S
DEPTH = 8       # hist ring slots per direction
ABLK = 4        # steps per acc block-add
PBLK = 2        # steps per psum proj block
CH = 512        # output-phase chunk (positions)
NCH = HW // CH


def _mkap(t, off, dims):
    """Custom free-dim AP on a tile: dims = [[step, count], ...] (outer->inner),
    off in elements of the tile's free space."""
    a = t[:]
    return bass.AP(a.tensor, a.offset + off, [list(a.ap[0])] + [list(d) for d in dims])


def _dram_ap(d, off, dims):
    a = d[:] if not isinstance(d, bass.AP) else d
    return bass.AP(a.tensor, off, [list(x) for x in dims])


def build_program(beta, use_kbias):
    nc = bacc.Bacc("TRN2", target_bir_lowering=False, debug=False)

    # ---- DRAM I/O ----
    x8_d = nc.dram_tensor("x8", [NK, 128, HW], DF8, kind="ExternalInput")
    xlo8_d = nc.dram_tensor("xlo8", [NK, 128, HW], DF8, kind="ExternalInput")
    xb_d = nc.dram_tensor("xb", [C, S, S], DBF, kind="ExternalInput")
    bl_d = nc.dram_tensor("bl", [S, S], F32, kind="ExternalInput")
    wi8_d = nc.dram_tensor("wi8", [NK, NK, 4, 128, 128], DF8, kind="ExternalInput")
    wilo8_d = nc.dram_tensor("wilo8", [NK, NK, 4, 128, 128], DF8, kind="ExternalInput")
    ws_d = nc.dram_tensor("ws_t", [4, C, C], DBF, kind="ExternalInput")
    wo_d = nc.dram_tensor("wo_t", [C, C], DBF, kind="ExternalInput")
    ob_d = nc.dram_tensor("ob", [C], F32, kind="ExternalInput")
    gnw_d = nc.dram_tensor("gnw", [C], F32, kind="ExternalInput")
    gnb_d = nc.dram_tensor("gnb", [C], F32, kind="ExternalInput")
    gsel_d = nc.dram_tensor("gsel", [2, 128, GN_GROUPS], F32, kind="ExternalInput")
    gexp_d = nc.dram_tensor("gexp", [2, GN_GROUPS, 128], F32, kind="ExternalInput")
    id_d = nc.dram_tensor("ident", [128, 128], F32, kind="ExternalInput")
    rid_d = nc.dram_tensor("revid", [128, 128], F32, kind="ExternalInput")
    idb_d = nc.dram_tensor("identb", [128, 128], DBF, kind="ExternalInput")
    if use_kbias:
        kb_d = nc.dram_tensor("kb", [4, C], DBF, kind="ExternalInput")
    gd = nc.dram_tensor("gdram", [S, 4, S], DBF)
    out_d = nc.dram_tensor("out", [C, S, S], DBF, kind="ExternalOutput")

    with tile.TileContext(nc) as tc:
        with ExitStack() as ctx:
            cp = ctx.enter_context(tc.tile_pool(name="const", bufs=1))

            # ---- resident (whole-program) tensors ----
            acc = [cp.tile([128, HW], DBF, tag=f"acc{k}", name=f"acc{k}") for k in range(NK)]
            ws = [cp.tile([128, 4, NK, 128], DBF, tag=f"ws{k}", name=f"ws{k}") for k in range(NK)]
            wo = [cp.tile([128, NK, 128], DBF, tag=f"wo{k}", name=f"wo{k}") for k in range(NK)]
            idb = cp.tile([128, 128], DBF, tag="idb")
            ob = [cp.tile([128, 1], F32, tag=f"ob{k}", name=f"ob{k}") for k in range(NK)]
            gnw = [cp.tile([128, 1], F32, tag=f"gnw{k}", name=f"gnw{k}") for k in range(NK)]
            gnb = [cp.tile([128, 1], F32, tag=f"gnb{k}", name=f"gnb{k}") for k in range(NK)]
            gsel = [cp.tile([128, GN_GROUPS], F32, tag=f"gsel{k}", name=f"gsel{k}") for k in range(NK)]
            gexp = [cp.tile([GN_GROUPS, 128], F32, tag=f"gexp{k}", name=f"gexp{k}") for k in range(NK)]
            sums = [cp.tile([128, NCH], F32, tag=f"sums{k}", name=f"sums{k}") for k in range(NK)]
            sumsq = [cp.tile([128, NCH], F32, tag=f"sumsq{k}", name=f"sumsq{k}") for k in range(NK)]
            epsb = cp.tile([GN_GROUPS, 1], F32, tag="epsb")

            with ExitStack() as sctx:
                # ---- scan-scoped tensors ----
                sp = sctx.enter_context(tc.tile_pool(name="scan", bufs=1))
                x8 = sp.tile([128, NK, HW], DF8, tag="x8")
                xlo8 = sp.tile([128, NK, HW], DF8, tag="xlo8")
                hist = sp.tile([128, NK, 4, DEPTH, S], DBF, tag="hist")
                wi8 = sp.tile([128, NK, 4, NK, 128], DF8, tag="wi8")
                wilo8 = sp.tile([128, NK, 4, NK, 128], DF8, tag="wilo8")
                G = sp.tile([128, 4, S], DBF, tag="G")
                bl = sp.tile([128, S], F32, tag="bl")
                conf = sp.tile([128, S], F32, tag="conf")
                confT = sp.tile([128, S], F32, tag="confT")
                confTr = sp.tile([128, S], F32, tag="confTr")
                confr = sp.tile([128, S], F32, tag="confr")
                ident = sp.tile([128, 128], F32, tag="ident")
                revid = sp.tile([128, 128], F32, tag="revid")
                s1 = sp.tile([128, S], F32, tag="s1")
                ngam = sp.tile([128, 1], F32, tag="ngam")
                if use_kbias:
                    kb = sp.tile([1, 4 * C], DBF, tag="kb")
                    ones_row = sp.tile([1, PBLK * S], DBF, tag="ones_row")
                    nc.vector.memset(ones_row[:], 1.0)

                # ---- gate computation (prioritized: it gates the scan) ----
                with tc.high_priority():
                    nc.sync.dma_start(bl[:, :], _dram_ap(bl_d, 0, [[S, 128], [1, S]]))
                    nc.sync.dma_start(ident[:], id_d[:])
                    nc.sync.dma_start(revid[:], rid_d[:])
                ctx_g = tc.high_priority()
                ctx_g.__enter__()
                nc.vector.memset(ngam[:], -GAMMA)
                nc.vector.memset(epsb[:], EPS)
                nc.scalar.activation(s1[:], bl[:], AF.Sigmoid)
                nc.scalar.activation(conf[:], s1[:], AF.Sigmoid, bias=ngam[:, 0:1],
                                     scale=ALPHA)
                nc.vector.tensor_scalar(conf[:], conf[:], -float(beta), 1.0, OP.mult, OP.add)
                nc.vector.tensor_scalar(conf[:], conf[:], 0.0, 1.0, OP.max, OP.min)

                # transposes: confT[t,n]=conf[n,t]; reversals via revid.
                with tc.tile_pool(name="tp_ps", bufs=1, space="PSUM") as tps:
                    pt = tps.tile([128, 128], F32, tag="pt")
                    nc.tensor.transpose(pt[:], conf[:], ident[:])
                    nc.vector.tensor_copy(confT[:], pt[:])
                    pt2 = tps.tile([128, 128], F32, tag="pt2")
                    nc.tensor.matmul(pt2[:], revid[:], conf[:], start=True, stop=True)
                    nc.vector.tensor_copy(confr[:], pt2[:])
                    pt3 = tps.tile([128, 128], F32, tag="pt3")
                    nc.tensor.matmul(pt3[:], revid[:], confT[:], start=True, stop=True)
                    nc.vector.tensor_copy(confTr[:], pt3[:])

                # G[t, dir, n] (bf16): lr=confT, rl=confTr, tb=conf, bt=confr
                nc.vector.tensor_copy(G[:, 0, :], confT[:])
                nc.vector.tensor_copy(G[:, 1, :], confTr[:])
                nc.vector.tensor_copy(G[:, 2, :], conf[:])
                nc.vector.tensor_copy(G[:, 3, :], confr[:])
                nc.sync.dma_start(_dram_ap(gd, 0, [[4 * S, 128], [1, 4 * S]]), G[:])
                ctx_g.__exit__(None, None, None)

                # ---- acc zero-init (off critical path; Pool + DVE split) ----
                nc.gpsimd.memset(acc[0][:], 0.0)
                for q in range(4):
                    nc.vector.memset(acc[1][:, q * (HW // 4):(q + 1) * (HW // 4)], 0.0)

                # ---- bulk DMAs in ----
                for k in range(NK):
                    for i in range(4):
                        nc.sync.dma_start(
                            ws[k][:, i, :, :],
                            _dram_ap(ws_d, i * C * C + k * 128 * C,
                                     [[C, 128], [128, NK], [1, 128]]))
                    nc.sync.dma_start(
                        wo[k][:], _dram_ap(wo_d, k * 128 * C,
                                           [[C, 128], [128, NK], [1, 128]]))
                    nc.sync.dma_start(ob[k][:], _dram_ap(ob_d, k * 128, [[1, 128], [1, 1]]))
                    nc.sync.dma_start(gnw[k][:], _dram_ap(gnw_d, k * 128, [[1, 128], [1, 1]]))
                    nc.sync.dma_start(gnb[k][:], _dram_ap(gnb_d, k * 128, [[1, 128], [1, 1]]))
                    nc.sync.dma_start(gsel[k][:], _dram_ap(gsel_d, k * 128 * GN_GROUPS,
                                                           [[GN_GROUPS, 128], [1, GN_GROUPS]]))
                    nc.sync.dma_start(gexp[k][:], _dram_ap(gexp_d, k * GN_GROUPS * 128,
                                                           [[128, GN_GROUPS], [1, 128]]))
                nc.sync.dma_start(idb[:], idb_d[:])
                # fp8 weights: [m, kt, d, kp, mc] -> tile [kp, kt, d, m, mc]
                for m in range(NK):
                    nc.sync.dma_start(
                        wi8[:, :, :, m, :],
                        _dram_ap(wi8_d, m * (NK * 4 * 128 * 128),
                                 [[128, 128], [4 * 128 * 128, NK], [128 * 128, 4], [1, 128]]))
                    nc.sync.dma_start(
                        wilo8[:, :, :, m, :],
                        _dram_ap(wilo8_d, m * (NK * 4 * 128 * 128),
                                 [[128, 128], [4 * 128 * 128, NK], [128 * 128, 4], [1, 128]]))
                if use_kbias:
                    nc.sync.dma_start(kb[:], _dram_ap(kb_d, 0, [[4 * C, 1], [1, 4 * C]]))
                # x8/xlo8: [kt, kp, pos] -> tile [kp, kt, pos]; chunked
                NXC = 4
                for j in range(NXC):
                    sz = HW // NXC
                    for t8, d8 in ((x8, x8_d), (xlo8, xlo8_d)):
                        nc.sync.dma_start(
                            t8[:, :, j * sz:(j + 1) * sz],
                            _dram_ap(d8, j * sz,
                                     [[HW, 128], [128 * HW, NK], [1, sz]]))

                # ================= SCAN =================
                # dirs: 0=lr, 1=rl, 2=tb, 3=bt
                def proj_rhs_blk(src, d, t0):
                    # DoubleRow rhs: free dims (ktile, step, n)
                    if d == 0:
                        return _mkap(src, t0, [[HW, NK], [1, PBLK], [S, S]])
                    return _mkap(src, t0 * S, [[HW, NK], [S, PBLK], [1, S]])

                def proj_rhs_step(src, d, t):
                    if d == 1:
                        return _mkap(src, S - 1 - t, [[HW, NK], [S, S]])
                    return _mkap(src, (S - 1 - t) * S, [[HW, NK], [1, S]])

                gbp = sctx.enter_context(tc.tile_pool(name="gb", bufs=6))
                hgp = sctx.enter_context(tc.tile_pool(name="hg", bufs=3))
                psp = sctx.enter_context(tc.tile_pool(name="ps", bufs=2, space="PSUM"))
                nblocks = S // PBLK
                ps_tiles = {}
                hg_cur = None

                def alloc_ps(b):
                    if b not in ps_tiles and b < nblocks:
                        ps_tiles[b] = psp.tile([128, NK, 4, PBLK * S], F32,
                                               tag="psb", name="psb")
                    return ps_tiles.get(b)

                terms = ((wi8, x8), (wi8, xlo8), (wilo8, x8))

                def emit_proj(b, m):
                    """fp8 DoubleRow input-proj matmuls for block b, half m."""
                    if b >= nblocks:
                        return
                    t0 = PBLK * b
                    ps = ps_tiles[b]
                    for d in range(4):
                        for ti_, (w8, src) in enumerate(terms):
                            lhsT = w8[:, :, d, m, :]
                            st = (d in (0, 2) and ti_ == 0)
                            if d in (0, 2):
                                nc.tensor.matmul(
                                    ps[:, m, d, :], lhsT, proj_rhs_blk(src, d, t0),
                                    start=st, stop=False, perf_mode=DR,
                                    skip_group_check=True)
                            else:
                                for ti in range(PBLK):
                                    nc.tensor.matmul(
                                        ps[:, m, d, ti * S:(ti + 1) * S],
                                        lhsT, proj_rhs_step(src, d, t0 + ti),
                                        start=False, stop=False, perf_mode=DR,
                                        skip_group_check=True)
                        if use_kbias:
                            nc.tensor.matmul(
                                ps[:, m, d, :],
                                kb[:, d * C + m * 128: d * C + (m + 1) * 128],
                                ones_row[:],
                                start=False, stop=False, skip_group_check=True)

                # prefetch all per-step gate rows; pool bufs self-pace the DMAs
                gb_tiles = {}
                for tt in range(1, S):
                    g_t = gbp.tile([128, 4, S], DBF, tag="gb", name="gb")
                    nc.sync.dma_start(
                        g_t[:], _dram_ap(gd, tt * 4 * S, [[0, 128], [1, 4 * S]]))
                    gb_tiles[tt] = g_t

                alloc_ps(0)
                for m in range(NK):
                    emit_proj(0, m)
                for b in range(nblocks):
                    t0 = PBLK * b
                    ps = ps_tiles.pop(b)
                    alloc_ps(b + 1)
                    for ti in range(PBLK):
                        t = t0 + ti
                        slot = t % DEPTH
                        gb = gb_tiles.pop(t + 1, None)
                        if gb is not None:
                            hg_nxt = hgp.tile([128, NK, 4, S], DBF, tag="hg", name="hg")
                        # per-direction: state matmuls -> evac -> hg mult
                        for d in range(4):
                            if t > 0:
                                for m in range(NK):
                                    for k in range(NK):
                                        nc.tensor.matmul(
                                            ps[:, m, d, ti * S:(ti + 1) * S],
                                            ws[k][:, d, m, :],
                                            hg_cur[:, k, d, :],
                                            start=False,
                                            stop=(ti == PBLK - 1 and k == NK - 1
                                                  and d % 2 == 1),
                                            skip_group_check=True)
                            # evac h = relu(ps) into hist ring (both m halves)
                            hslot = hist[:, :, d, slot, :]
                            pslice = ps[:, :, d, ti * S:(ti + 1) * S]
                            if d < 3:
                                nc.scalar.activation(hslot, pslice, AF.Relu)
                            else:
                                nc.vector.tensor_scalar(hslot, pslice, 0.0, None, OP.max)
                            # hg = h * g (bf16 SBUF, 2x mode); gate row t+1
                            if gb is not None:
                                gb_b = _mkap(gb, d * S, [[0, NK], [1, S]])
                                nc.vector.tensor_tensor(
                                    hg_nxt[:, :, d, :], hist[:, :, d, slot, :],
                                    gb_b, OP.mult)
                        if gb is not None:
                            hg_cur = hg_nxt
                        # next block's proj fills the PE gap
                        emit_proj(b + 1, ti)
                        # ---- acc block adds every ABLK steps ----
                        if t % ABLK == ABLK - 1:
                            tb0 = t - (ABLK - 1)
                            s0 = tb0 % DEPTH
                            for k in range(NK):
                                hoff = k * (4 * DEPTH * S)
                                # lr: cols tb0..t (gpsimd)
                                nc.gpsimd.tensor_tensor(
                                    _mkap(acc[k], tb0, [[S, S], [1, ABLK]]),
                                    _mkap(acc[k], tb0, [[S, S], [1, ABLK]]),
                                    _mkap(hist, hoff + (0 * DEPTH + s0) * S,
                                          [[1, S], [S, ABLK]]),
                                    OP.add)
                                # rl: cols S-1-tb0 down
                                nc.gpsimd.tensor_tensor(
                                    _mkap(acc[k], S - 1 - tb0, [[S, S], [-1, ABLK]]),
                                    _mkap(acc[k], S - 1 - tb0, [[S, S], [-1, ABLK]]),
                                    _mkap(hist, hoff + (1 * DEPTH + s0) * S,
                                          [[1, S], [S, ABLK]]),
                                    OP.add)
                                # tb: rows tb0..t (DVE, packed)
                                nc.vector.tensor_tensor(
                                    _mkap(acc[k], tb0 * S, [[S, ABLK], [1, S]]),
                                    _mkap(acc[k], tb0 * S, [[S, ABLK], [1, S]]),
                                    hist[:, k, 2, s0:s0 + ABLK, :],
                                    OP.add)
                                # bt: rows S-1-tb0 down
                                nc.vector.tensor_tensor(
                                    _mkap(acc[k], (S - 1 - tb0) * S, [[-S, ABLK], [1, S]]),
                                    _mkap(acc[k], (S - 1 - tb0) * S, [[-S, ABLK], [1, S]]),
                                    hist[:, k, 3, s0:s0 + ABLK, :],
                                    OP.add)

            # ================= OUTPUT =================
            inv_n = 1.0 / (8.0 * HW)
            with ExitStack() as octx:
                op_ = octx.enter_context(tc.tile_pool(name="oconst", bufs=1))
                z = op_.tile([128, NK, HW], DBF, tag="z")
                ops_pool = octx.enter_context(tc.tile_pool(name="ops", bufs=2, space="PSUM"))
                oxr = octx.enter_context(tc.tile_pool(name="oxr", bufs=3))
                ojk = octx.enter_context(tc.tile_pool(name="ojk", bufs=2))
                ostp = octx.enter_context(tc.tile_pool(name="ost", bufs=1))
                obp = octx.enter_context(tc.tile_pool(name="obuf", bufs=3))

                # ---- pass 1: z = Wo@acc + ob + x, stats via accum_out ----
                xr_tiles = {}
                for j in range(NCH):
                    xr = oxr.tile([128, NK, CH], DBF, tag="xr", name="xr")
                    nc.sync.dma_start(
                        xr[:], _dram_ap(xb_d, j * CH,
                                        [[HW, 128], [128 * HW, NK], [1, CH]]))
                    xr_tiles[j] = xr
                for j in range(NCH):
                    xr = xr_tiles.pop(j)
                    pso = ops_pool.tile([128, NK, CH], F32, tag="pso", name="pso")
                    for m in range(NK):
                        for k in range(NK):
                            nc.tensor.matmul(pso[:, m, :], wo[k][:, m, :],
                                             acc[k][:, j * CH:(j + 1) * CH],
                                             start=(k == 0), stop=False,
                                             skip_group_check=True)
                        # residual folded in via identity matmul
                        nc.tensor.matmul(pso[:, m, :], idb[:], xr[:, m, :],
                                         start=False, stop=True,
                                         skip_group_check=True)
                    for m in range(NK):
                        zjm = z[:, m, j * CH:(j + 1) * CH]
                        nc.scalar.activation(zjm, pso[:, m, :], AF.Identity,
                                             bias=ob[m][:, 0:1],
                                             accum_out=sums[m][:, j:j + 1])
                        junk = ojk.tile([128, CH], DBF, tag=f"junk{m}",
                                        name=f"junk{m}")
                        nc.vector.scalar_tensor_tensor(
                            junk[:], zjm, 0.0, zjm, OP.add, OP.mult,
                            accum_out=sumsq[m][:, j:j + 1])

                # ---- group stats ----
                ssq = [ostp.tile([128, 2], F32, tag=f"ssq{k}", name=f"ssq{k}") for k in range(NK)]
                for k in range(NK):
                    nc.vector.tensor_reduce(ssq[k][:, 0:1], sums[k][:, 0:NCH],
                                            mybir.AxisListType.X, OP.add)
                    nc.vector.tensor_reduce(ssq[k][:, 1:2], sumsq[k][:, 0:NCH],
                                            mybir.AxisListType.X, OP.add)
                with tc.tile_pool(name="stps", bufs=1, space="PSUM") as stps:
                    psg = stps.tile([GN_GROUPS, 2], F32, tag="psg")
                    for k in range(NK):
                        nc.tensor.matmul(psg[:], gsel[k][:], ssq[k][:],
                                         start=(k == 0), stop=(k == NK - 1))
                    mv = ostp.tile([GN_GROUPS, 2], F32, tag="mv")
                    nc.vector.tensor_scalar(mv[:], psg[:], inv_n, None, OP.mult)
                    mu2 = ostp.tile([GN_GROUPS, 1], F32, tag="mu2")
                    nc.vector.tensor_tensor(mu2[:], mv[:, 0:1], mv[:, 0:1], OP.mult)
                    var = ostp.tile([GN_GROUPS, 1], F32, tag="var")
                    nc.vector.tensor_tensor(var[:], mv[:, 1:2], mu2[:], OP.subtract)
                    sd = ostp.tile([GN_GROUPS, 1], F32, tag="sd")
                    nc.scalar.activation(sd[:], var[:], AF.Sqrt, bias=epsb[:, 0:1])
                    rstd = ostp.tile([GN_GROUPS, 1], F32, tag="rstd")
                    nc.vector.reciprocal(rstd[:], sd[:])
                    mr = ostp.tile([GN_GROUPS, 2], F32, tag="mr")
                    nc.vector.tensor_copy(mr[:, 0:1], mv[:, 0:1])
                    nc.vector.tensor_copy(mr[:, 1:2], rstd[:])
                    scale = [ostp.tile([128, 1], F32, tag=f"scale{k}", name=f"scale{k}") for k in range(NK)]
                    bias = [ostp.tile([128, 1], F32, tag=f"bias{k}", name=f"bias{k}") for k in range(NK)]
                    for k in range(NK):
                        pse = stps.tile([128, 2], F32, tag=f"pse{k}")
                        nc.tensor.matmul(pse[:], gexp[k][:], mr[:], start=True, stop=True)
                        muc = ostp.tile([128, 1], F32, tag=f"muc{k}")
                        rc = ostp.tile([128, 1], F32, tag=f"rc{k}")
                        nc.vector.tensor_copy(muc[:], pse[:, 0:1])
                        nc.vector.tensor_copy(rc[:], pse[:, 1:2])
                        nc.vector.tensor_tensor(scale[k][:], rc[:], gnw[k][:], OP.mult)
                        tmp = ostp.tile([128, 1], F32, tag=f"tmp{k}")
                        nc.vector.tensor_tensor(tmp[:], muc[:], scale[k][:], OP.mult)
                        nc.vector.tensor_tensor(bias[k][:], gnb[k][:], tmp[:], OP.subtract)

                # ---- pass 2: of = z*scale + bias (DVE 4x), stream out ----
                for j in range(NCH):
                    of = obp.tile([128, NK, CH], DBF, tag="of", name="of")
                    for m in range(NK):
                        nc.vector.tensor_scalar(
                            of[:, m, :], z[:, m, j * CH:(j + 1) * CH],
                            scale[m][:, 0:1], bias[m][:, 0:1], OP.mult, OP.add)
                    nc.sync.dma_start(
                        _dram_ap(out_d, j * CH,
                                 [[HW, 128], [128 * HW, NK], [1, CH]]),
                        of[:])
    nc.compile()
    return nc


_CACHE = {}


def _get_program(beta, use_kbias):
    key = (float(beta), bool(use_kbias))
    if key not in _CACHE:
        _CACHE[key] = build_program(beta, use_kbias)
    return _CACHE[key]


def make_host_inputs(feature, boundary_logits, beta, W_in, b_in, W_s, b_s,
                     p_bias, out_w, out_b, gn_w, gn_b):
    f32 = np.float32
    W_in = np.asarray(W_in, f32)
    ws_t = np.ascontiguousarray(
        np.transpose(np.asarray(W_s, f32), (0, 2, 1))).astype(BF)
    wo_t = np.ascontiguousarray(np.asarray(out_w, f32).T).astype(BF)
    kbv = (np.asarray(b_in, f32) + np.asarray(b_s, f32)
           + np.asarray(p_bias, f32))
    use_kbias = bool(np.any(kbv != 0.0))

    # fp8 weight splits: wi8[m, kt, d, kp, mc] = f8(W_in[d, m*128+mc, kt*128+kp])
    wi_t = np.transpose(W_in, (0, 2, 1))  # [d, cin, cout] fp32
    w8 = wi_t.astype(F8)
    wlo8 = (wi_t - w8.astype(f32)).astype(F8)

    def pack_w(w):  # [d, cin, cout] -> [m, kt, d, kp, mc]
        r = w.reshape(4, NK, 128, NK, 128)        # [d, kt, kp, m, mc]
        return np.ascontiguousarray(r.transpose(3, 1, 0, 2, 4))

    wi8_np = pack_w(w8)
    wilo8_np = pack_w(wlo8)

    cpg = C // GN_GROUPS
    gsel = np.zeros((2, 128, GN_GROUPS), f32)
    gexp = np.zeros((2, GN_GROUPS, 128), f32)
    for k in range(2):
        for p in range(128):
            g = (k * 128 + p) // cpg
            gsel[k, p, g] = 1.0
            gexp[k, g, p] = 1.0
    common = {
        "wi8": wi8_np, "wilo8": wilo8_np, "ws_t": ws_t, "wo_t": wo_t,
        "ob": np.asarray(out_b, f32),
        "gnw": np.asarray(gn_w, f32),
        "gnb": np.asarray(gn_b, f32),
        "gsel": gsel, "gexp": gexp,
        "ident": np.eye(128, dtype=f32),
        "revid": np.eye(128, dtype=f32)[::-1].copy(),
        "identb": np.eye(128, dtype=f32).astype(BF),
    }
    if use_kbias:
        common["kb"] = kbv.astype(BF)
    B = np.asarray(feature).shape[0]
    in_maps = []
    for b in range(B):
        m = dict(common)
        xf = np.asarray(feature[b], f32).reshape(C, HW)
        x8 = xf.astype(F8)
        xlo8 = (xf - x8.astype(f32)).astype(F8)
        m["x8"] = np.ascontiguousarray(x8.reshape(NK, 128, HW))
        m["xlo8"] = np.ascontiguousarray(xlo8.reshape(NK, 128, HW))
        m["xb"] = xf.reshape(C, S, S).astype(BF)
        m["bl"] = np.asarray(boundary_logits[b], f32).reshape(S, S)
        in_maps.append(m)
    return in_maps, float(np.asarray(beta).reshape(-1)[0]), use_kbias


def kernel(feature, boundary_logits, beta, W_in, b_in, W_s, b_s, p_bias,
           out_w, out_b, gn_w, gn_b):
    feature = np.asarray(feature)
    B = feature.shape[0]
    in_maps, beta_v, use_kbias = make_host_inputs(
        feature, boundary_logits, beta, W_in, b_in, W_s, b_s, p_bias,
        out_w, out_b, gn_w, gn_b)
    nc = _get_program(beta_v, use_kbias)
    res = run_bass_kernel_spmd(nc, in_maps, core_ids=list(range(B)))
    out = np.stack([np.asarray(r["out"], np.float32) for r in res.results], axis=0)
    return out.reshape(B, C, S, S)
